# revision 1
# baseline (speedup 1.0000x reference)
"""Ternary-weight linear layer on 8 Trainium2 NeuronCores.

Problem: y = x @ ternarize(W).T + b
  x [8192, 4096] fp32, W [4096, 4096] fp32, b [4096] fp32.
  ternarize(w) = round(clamp(w, -1, 1))  (round-half-even, forward value).

Strategy (data-parallel over tokens, replicated weights):
  - Each of the 8 cores gets 1024 tokens. Host passes x and W already
    transposed (pure layout prep) so the contraction dim i lands on SBUF
    partitions with no on-device transposes:
        xT  [4096 i, 1024 t]  (per-core slice)
        wT  [4096 i, 4096 o]  (replicated)
  - On device, W tiles are ternarized exactly with two chained DVE
    tensor_scalar ops: clamp via min/max, then round-half-even via the
    +C/-C trick (C = 1.5 * 2^23). Ternary values are exact in bf16.
  - mode "bf16x2": x is split on-device into x_hi = bf16(x) and
    x_lo = bf16(x - x_hi); two bf16 matmuls accumulate into the same
    PSUM bank. bf16 streams 1 cycle/column on the PE (measured 198
    ns per 512-col matmul) and exact ternary weights make the result
    accurate to ~2e-6 relative.
  - mode "f32r": single-pass float32r matmuls (measured 434 ns/MM =
    2 cycles/column, ~1e-4 relative error). Same speed as bf16x2 but
    less accurate; kept for comparison.
  - Bias is added during PSUM->SBUF eviction on the scalar engine
    (activation Identity with per-partition bias).
  - Per-core output is yT [4096 o, 1024 t]; the host transposes and
    concatenates (layout-only unshard).
"""

import numpy as np

N_CORES = 8
TOKENS = 8192
IN_F = 4096
OUT_F = 4096
T_CORE = TOKENS // N_CORES       # 1024 tokens per core
P = 128                          # partitions
KB = IN_F // P                   # 32 contraction blocks
TN = 512                         # moving free dim per matmul (1 PSUM bank)
TH = T_CORE // TN                # 2 t-halves
O_CHUNK = 256                    # o columns ternarized/matmul'd per pass
OB_PER_CHUNK = O_CHUNK // P      # 2
N_CHUNKS = OUT_F // O_CHUNK      # 16

C_ROUND = 12582912.0             # 1.5 * 2^23; (x+C)-C == round-half-even(x), |x|<=1

MODE = "f32r"                    # "bf16x2" | "f32r"

_built = None


def _build(reps=1, mode=MODE, o_chunk=O_CHUNK, wbufs=6, obufs=3, ps_bufs=2):
    import contextlib
    import concourse.bacc as bacc
    import concourse.mybir as mybir
    import concourse.tile as tile

    dt = mybir.dt
    x_in_dt = dt.float32 if mode == "bf16x2" else dt.float32r
    w_dt = dt.bfloat16 if mode == "bf16x2" else dt.float32r

    nc = bacc.Bacc("TRN2", target_bir_lowering=False, debug=False)
    xT_d = nc.dram_tensor("xT", [IN_F, T_CORE], x_in_dt, kind="ExternalInput").ap()
    wT_d = nc.dram_tensor("wT", [IN_F, OUT_F], dt.float32, kind="ExternalInput").ap()
    biasT_d = nc.dram_tensor("biasT", [P, OUT_F // P], dt.float32, kind="ExternalInput").ap()
    yT_d = nc.dram_tensor("yT", [OUT_F, T_CORE], dt.float32, kind="ExternalOutput").ap()

    with tile.TileContext(nc) as tc:
        with tc.tile_pool(name="xp", bufs=1) as xp, \
             tc.tile_pool(name="xi", bufs=3) as xi, \
             tc.tile_pool(name="wp", bufs=wbufs) as wp, \
             tc.tile_pool(name="wc", bufs=max(2, wbufs - 1)) as wc, \
             tc.tile_pool(name="wt", bufs=wbufs) as wtp, \
             tc.tile_pool(name="op", bufs=obufs) as op, \
             tc.tile_pool(name="cn", bufs=1) as cn, \
             tc.tile_pool(name="ps", bufs=ps_bufs, space="PSUM") as ps:

            biasT = cn.tile([P, OUT_F // P], dt.float32, name="biasT_s")
            nc.sync.dma_start(out=biasT[:], in_=biasT_d[:])

            # x resident in SBUF
            xsrc = []          # list of (hi, lo) or single fp32r tiles
            for kb in range(KB):
                sl = xT_d[kb * P:(kb + 1) * P, :]
                if mode == "bf16x2":
                    xtmp = xi.tile([P, T_CORE], dt.float32, tag="xtmp",
                                   name=f"xtmp{kb}")
                    nc.sync.dma_start(out=xtmp[:], in_=sl)
                    xhi = xp.tile([P, T_CORE], dt.bfloat16, tag=f"xh{kb}",
                                  name=f"xh{kb}")
                    nc.vector.tensor_copy(xhi[:], xtmp[:])
                    xlo = xp.tile([P, T_CORE], dt.bfloat16, tag=f"xl{kb}",
                                  name=f"xl{kb}")
                    nc.vector.tensor_sub(xlo[:], xtmp[:], xhi[:])
                    xsrc.append((xhi, xlo))
                else:
                    t = xp.tile([P, T_CORE], dt.float32r, tag=f"x{kb}",
                                name=f"x{kb}")
                    # SWDGE queues: keeps the 16MB x prologue off the HWDGE
                    # rings so the first W-chunk DMAs aren't queued behind it
                    nc.gpsimd.dma_start(out=t[:], in_=sl)
                    xsrc.append((t,))

            ob_per_chunk = o_chunk // P
            n_chunks = OUT_F // o_chunk
            rep_ctx = tc.For_i(0, reps, 1) if reps > 1 else contextlib.nullcontext()
            with rep_ctx:
              for ch in range(n_chunks):
                o0 = ch * o_chunk
                psums = [
                    ps.tile([P, TN], dt.float32, tag=f"ps{ob}_{th}",
                            name=f"ps_{ch}_{ob}_{th}")
                    for ob in range(ob_per_chunk) for th in range(TH)
                ]
                for kb in range(KB):
                    wtile = wp.tile([P, o_chunk], dt.float32, tag="w",
                                    name=f"w_{ch}_{kb}")
                    nc.sync.dma_start(
                        out=wtile[:],
                        in_=wT_d[kb * P:(kb + 1) * P, o0:o0 + o_chunk])
                    wcl = wc.tile([P, o_chunk], dt.float32, tag="wcl",
                                  name=f"wcl_{ch}_{kb}")
                    nc.vector.tensor_scalar(wcl[:], wtile[:], 1.0, -1.0,
                                            mybir.AluOpType.min,
                                            mybir.AluOpType.max)
                    wter = wtp.tile([P, o_chunk], w_dt, tag="wter",
                                    name=f"wter_{ch}_{kb}")
                    nc.vector.tensor_scalar(wter[:], wcl[:], C_ROUND, C_ROUND,
                                            mybir.AluOpType.add,
                                            mybir.AluOpType.subtract)
                    first, last = kb == 0, kb == KB - 1
                    for ob in range(ob_per_chunk):
                        lhsT = wter[:, ob * P:(ob + 1) * P]
                        for th in range(TH):
                            for xi_, xpart in enumerate(xsrc[kb]):
                                nc.tensor.matmul(
                                    psums[ob * TH + th][:],
                                    lhsT,
                                    xpart[:, th * TN:(th + 1) * TN],
                                    start=(first and xi_ == 0),
                                    stop=(last and xi_ == len(xsrc[kb]) - 1))

                # evict PSUM -> SBUF with fused bias add, then DMA out
                for ob in range(ob_per_chunk):
                    o_abs = o0 + ob * P
                    stage = op.tile([P, T_CORE], dt.float32, tag="out",
                                    name=f"out_{ch}_{ob}")
                    for th in range(TH):
                        nc.scalar.activation(
                            stage[:, th * TN:(th + 1) * TN],
                            psums[ob * TH + th][:],
                            mybir.ActivationFunctionType.Identity,
                            bias=biasT[:, o_abs // P:o_abs // P + 1],
                            scale=1.0)
                    nc.sync.dma_start(
                        out=yT_d[o_abs:o_abs + P, :], in_=stage[:])

    nc.compile()
    return nc


def kernel(input, weight, bias):
    global _built
    if _built is None:
        _built = _build()
    nc = _built
    from concourse.bass_utils import run_bass_kernel_spmd

    input = np.ascontiguousarray(input, dtype=np.float32)
    weight = np.ascontiguousarray(weight, dtype=np.float32)
    bias = np.ascontiguousarray(bias, dtype=np.float32)

    wT = np.ascontiguousarray(weight.T)                      # [i, o]
    biasT = np.ascontiguousarray(bias.reshape(OUT_F // P, P).T)  # [128, 32]

    in_maps = []
    for c in range(N_CORES):
        x_c = input[c * T_CORE:(c + 1) * T_CORE]             # [1024, 4096]
        xT_c = np.ascontiguousarray(x_c.T)                   # [4096, 1024]
        in_maps.append({"xT": xT_c, "wT": wT, "biasT": biasT})

    res = run_bass_kernel_spmd(nc, in_maps, list(range(N_CORES)))

    y = np.empty((TOKENS, OUT_F), dtype=np.float32)
    for c in range(N_CORES):
        y[c * T_CORE:(c + 1) * T_CORE] = res.results[c]["yT"].T
    return y



# revision 2
# speedup vs baseline: 1.5879x; 1.5879x over previous
"""Ternary-weight linear layer on 8 Trainium2 NeuronCores.

Problem: y = x @ ternarize(W).T + b
  x [8192, 4096] fp32, W [4096, 4096] fp32, b [4096] fp32.
  ternarize(w) = round(clamp(w, -1, 1))  (round-half-even, forward value).

Strategy (data-parallel over tokens, replicated weights, fp8 DoubleRow):
  - Each of the 8 cores gets 1024 tokens. Host passes x and W transposed
    (layout-only prep) so the contraction dim i lands on SBUF partitions:
        xT  [4096 i, 1024 t]  (per-core slice)
        wT  [4096 i, 4096 o]  (replicated)
  - W is ternarized exactly on device with two DVE tensor_scalar ops:
    clamp via min/max (f32), then round-half-even via +C/-C (C = 1.5*2^23)
    writing float8e4 directly ({-1, 0, 1} are exact in fp8).
  - x is split on device into x_hi = fp8(x) and x_lo = fp8(x - x_hi).
    Two fp8 matmul passes accumulate into the same PSUM region; combined
    x quantization error is ~2^-8 relative (well under tolerance).
  - Matmuls use MatmulPerfMode.DoubleRow: lhsT [128, 2, 128] fp8 and
    rhs [128, 2, 256] fp8 contract TWO 128-deep k-slabs per instruction
    at 0.5 cycles per output column - 2x the bf16/f32r rate.
  - PSUM: one 2KB bank [128, 512] f32 per (o-block, token-half) group;
    the two 256-token sub-regions accumulate within one start/stop group
    (PSUM zeroing granularity is the 2KB bank).
  - Bias is added during PSUM->SBUF eviction on the scalar engine; the
    output is written as bf16 (halves the y DMA traffic; |rel err| ~2^-9).
  - Per-core output yT [4096 o, 1024 t] bf16; host casts + transposes.
"""

import contextlib

import numpy as np

N_CORES = 8
TOKENS = 8192
IN_F = 4096
OUT_F = 4096
T_CORE = TOKENS // N_CORES       # 1024 tokens per core
P = 128                          # partitions
KB = IN_F // P                   # 32 contraction slabs of 128
NQ = 4                           # x token quarters
TQ = T_CORE // NQ                # 256 tokens per quarter / per matmul
O_CHUNK = 256                    # o columns per W chunk
N_CHUNKS = OUT_F // O_CHUNK      # 16
C_ROUND = 12582912.0             # 1.5 * 2^23; (v+C)-C == round-half-even(v)

_built = None


def _build(reps=1):
    import concourse.bacc as bacc
    import concourse.mybir as mybir
    import concourse.tile as tile

    dt = mybir.dt
    DR = mybir.MatmulPerfMode.DoubleRow

    nc = bacc.Bacc("TRN2", target_bir_lowering=False, debug=False)
    xT_d = nc.dram_tensor("xT", [IN_F, T_CORE], dt.float32, kind="ExternalInput").ap()
    wT_d = nc.dram_tensor("wT", [IN_F, OUT_F], dt.float32, kind="ExternalInput").ap()
    biasT_d = nc.dram_tensor("biasT", [P, OUT_F // P], dt.float32,
                             kind="ExternalInput").ap()
    yT_d = nc.dram_tensor("yT", [OUT_F, T_CORE], dt.bfloat16,
                          kind="ExternalOutput").ap()

    xT_r = xT_d.rearrange("(kb p) t -> p kb t", p=P)     # [128, 32, 1024]
    wT_r = wT_d.rearrange("(kb p) o -> p kb o", p=P)     # [128, 32, 4096]

    with tile.TileContext(nc) as tc:
        with tc.tile_pool(name="xq", bufs=1) as xq, \
             tc.tile_pool(name="xf", bufs=2) as xf, \
             tc.tile_pool(name="wf", bufs=2) as wfp, \
             tc.tile_pool(name="wq", bufs=2) as wqp, \
             tc.tile_pool(name="op", bufs=3) as op, \
             tc.tile_pool(name="cn", bufs=1) as cn, \
             tc.tile_pool(name="ps", bufs=2, space="PSUM") as ps:

            biasT = cn.tile([P, OUT_F // P], dt.float32, name="biasT_s")
            nc.sync.dma_start(out=biasT[:], in_=biasT_d[:])

            # x: DMA per token-quarter (ACT queue, so W DMAs on SP aren't
            # queued behind it), then split into resident fp8 hi/lo tiles.
            xhi, xlo = [], []
            for q in range(NQ):
                stage = xf.tile([P, KB, TQ], dt.float32, tag="xf",
                                name=f"xf{q}")
                nc.scalar.dma_start(out=stage[:],
                                    in_=xT_r[:, :, q * TQ:(q + 1) * TQ])
                hi = xq.tile([P, KB, TQ], dt.float8e4, tag=f"xh{q}",
                             name=f"xh{q}")
                nc.vector.tensor_copy(hi[:], stage[:])
                lo = xq.tile([P, KB, TQ], dt.float8e4, tag=f"xl{q}",
                             name=f"xl{q}")
                nc.vector.tensor_sub(lo[:], stage[:], hi[:])
                xhi.append(hi)
                xlo.append(lo)

            rep_ctx = tc.For_i(0, reps, 1) if reps > 1 else contextlib.nullcontext()
            with rep_ctx:
              for ch in range(N_CHUNKS):
                o0 = ch * O_CHUNK
                # W chunk: two half-chunk DMAs (512B descriptors), ternarize
                # in-place clamp then round into fp8.
                wq = wqp.tile([P, KB, O_CHUNK], dt.float8e4, tag="wq",
                              name=f"wq{ch}")
                for h in range(2):
                    wf = wfp.tile([P, KB, O_CHUNK // 2], dt.float32, tag="wf",
                                  name=f"wf{ch}_{h}")
                    lo_o = o0 + h * (O_CHUNK // 2)
                    nc.sync.dma_start(
                        out=wf[:], in_=wT_r[:, :, lo_o:lo_o + O_CHUNK // 2])
                    nc.vector.tensor_scalar(wf[:], wf[:], 1.0, -1.0,
                                            mybir.AluOpType.min,
                                            mybir.AluOpType.max)
                    nc.vector.tensor_scalar(
                        wq[:, :, h * (O_CHUNK // 2):(h + 1) * (O_CHUNK // 2)],
                        wf[:], C_ROUND, C_ROUND,
                        mybir.AluOpType.add, mybir.AluOpType.subtract)

                stages = [
                    op.tile([P, T_CORE], dt.bfloat16, tag=f"out{ob}",
                            name=f"out_{ch}_{ob}")
                    for ob in range(O_CHUNK // P)
                ]
                for th in range(2):            # token halves (512 each)
                    for ob in range(O_CHUNK // P):
                        o_abs = o0 + ob * P
                        lhs_lo = ob * P
                        psum = ps.tile([P, 512], dt.float32, tag=f"ps{ob}{th}",
                                       name=f"ps_{ch}_{ob}_{th}")
                        for tl, tb in enumerate((2 * th, 2 * th + 1)):
                            for s in range(KB // 2):
                                for pi, xsrc in enumerate((xhi[tb], xlo[tb])):
                                    nc.tensor.matmul(
                                        psum[:, tl * TQ:(tl + 1) * TQ],
                                        wq[:, 2 * s:2 * s + 2,
                                           lhs_lo:lhs_lo + P],
                                        xsrc[:, 2 * s:2 * s + 2, :],
                                        start=(tl == 0 and s == 0 and pi == 0),
                                        stop=(tl == 1 and s == KB // 2 - 1
                                              and pi == 1),
                                        perf_mode=DR)
                        nc.scalar.activation(
                            stages[ob][:, th * 512:(th + 1) * 512],
                            psum[:],
                            mybir.ActivationFunctionType.Identity,
                            bias=biasT[:, o_abs // P:o_abs // P + 1],
                            scale=1.0)
                for ob in range(O_CHUNK // P):
                    o_abs = o0 + ob * P
                    nc.scalar.dma_start(out=yT_d[o_abs:o_abs + P, :],
                                        in_=stages[ob][:])

    nc.compile()
    return nc


def kernel(input, weight, bias):
    global _built
    if _built is None:
        _built = _build()
    nc = _built
    from concourse.bass_utils import run_bass_kernel_spmd

    input = np.ascontiguousarray(input, dtype=np.float32)
    weight = np.ascontiguousarray(weight, dtype=np.float32)
    bias = np.ascontiguousarray(bias, dtype=np.float32)

    wT = np.ascontiguousarray(weight.T)                          # [i, o]
    biasT = np.ascontiguousarray(bias.reshape(OUT_F // P, P).T)  # [128, 32]

    in_maps = []
    for c in range(N_CORES):
        x_c = input[c * T_CORE:(c + 1) * T_CORE]                 # [1024, 4096]
        xT_c = np.ascontiguousarray(x_c.T)                       # [4096, 1024]
        in_maps.append({"xT": xT_c, "wT": wT, "biasT": biasT})

    res = run_bass_kernel_spmd(nc, in_maps, list(range(N_CORES)))

    y = np.empty((TOKENS, OUT_F), dtype=np.float32)
    for c in range(N_CORES):
        y[c * T_CORE:(c + 1) * T_CORE] = \
            np.asarray(res.results[c]["yT"]).astype(np.float32).T
    return y


# revision 8
# speedup vs baseline: 1.6496x; 1.0388x over previous
"""Ternary-weight linear layer on 8 Trainium2 NeuronCores.

Problem: y = x @ ternarize(W).T + b
  x [8192, 4096] fp32, W [4096, 4096] fp32, b [4096] fp32.
  ternarize(w) = round(clamp(w, -1, 1))  (round-half-even, forward value).

Strategy (data-parallel over tokens, replicated weights, fp8 DoubleRow):
  - Each of the 8 cores gets 1024 tokens. Host passes x and W transposed
    (layout-only prep) so the contraction dim i lands on SBUF partitions:
        xT  [4096 i, 1024 t]  (per-core slice)
        wT  [4096 i, 4096 o]  (replicated)
  - W is ternarized exactly on device with two DVE tensor_scalar ops:
    clamp via min/max (f32), then round-half-even via +C/-C (C = 1.5*2^23)
    writing float8e4 directly ({-1, 0, 1} are exact in fp8).
  - x is split on device into x_hi = fp8(x) and x_lo = fp8(x - x_hi).
    Two fp8 matmul passes accumulate into the same PSUM region; combined
    x quantization error is ~2^-8 relative (well under tolerance).
  - Matmuls use MatmulPerfMode.DoubleRow: lhsT [128, 2, 128] fp8 and
    rhs [128, 2, 256] fp8 contract TWO 128-deep k-slabs per instruction
    at 0.5 cycles per output column - 2x the bf16/f32r rate.
  - Pipeline: x quarters and the first W chunks are interleaved on one
    DMA queue (and in DVE issue order) so the PE starts early; each
    (chunk, o-block, token-quarter) cell accumulates in its own PSUM
    bank (8 banks rotating) and is evicted as soon as it stops.
  - Bias is added during PSUM->SBUF eviction on the scalar engine; the
    output is written as bf16 (halves the y DMA traffic; |rel err| ~2^-9).
  - Per-core output yT [4096 o, 1024 t] bf16; host casts + transposes.
"""

import contextlib

import numpy as np

N_CORES = 8
TOKENS = 8192
IN_F = 4096
OUT_F = 4096
T_CORE = TOKENS // N_CORES       # 1024 tokens per core
P = 128                          # partitions
KB = IN_F // P                   # 32 contraction slabs of 128
NQ = 4                           # x token quarters
TQ = T_CORE // NQ                # 256 tokens per quarter / per matmul
O_CHUNK = 256                    # o columns per W chunk
N_CHUNKS = OUT_F // O_CHUNK      # 16
C_ROUND = 12582912.0             # 1.5 * 2^23; (v+C)-C == round-half-even(v)

_built = None


def _build(reps=1):
    import concourse.bacc as bacc
    import concourse.mybir as mybir
    import concourse.tile as tile

    dt = mybir.dt
    DR = mybir.MatmulPerfMode.DoubleRow

    nc = bacc.Bacc("TRN2", target_bir_lowering=False, debug=False)
    xT_d = nc.dram_tensor("xT", [IN_F, T_CORE], dt.float32, kind="ExternalInput").ap()
    wT_d = nc.dram_tensor("wT", [IN_F, OUT_F], dt.float32, kind="ExternalInput").ap()
    biasT_d = nc.dram_tensor("biasT", [P, OUT_F // P], dt.float32,
                             kind="ExternalInput").ap()
    yT_d = nc.dram_tensor("yT", [OUT_F, T_CORE], dt.bfloat16,
                          kind="ExternalOutput").ap()

    xT_r = xT_d.rearrange("(kb p) t -> p kb t", p=P)     # [128, 32, 1024]
    wT_r = wT_d.rearrange("(kb p) o -> p kb o", p=P)     # [128, 32, 4096]

    with tile.TileContext(nc) as tc:
        with tc.tile_pool(name="xq", bufs=1) as xq, \
             tc.tile_pool(name="xf", bufs=2) as xf, \
             tc.tile_pool(name="wf", bufs=2) as wfp, \
             tc.tile_pool(name="wq", bufs=2) as wqp, \
             tc.tile_pool(name="op", bufs=8) as op, \
             tc.tile_pool(name="cn", bufs=1) as cn, \
             tc.tile_pool(name="ps", bufs=8, space="PSUM") as ps:

            biasT = cn.tile([P, OUT_F // P], dt.float32, name="biasT_s")
            nc.sync.dma_start(out=biasT[:], in_=biasT_d[:])

            xhi, xlo = [None] * NQ, [None] * NQ

            def load_quarter_hi(q):
                """DMA one x token-quarter and convert the fp8 hi part."""
                stage = xf.tile([P, KB, TQ], dt.float32, tag="xf",
                                name=f"xf{q}")
                nc.sync.dma_start(out=stage[:],
                                  in_=xT_r[:, :, q * TQ:(q + 1) * TQ])
                hi = xq.tile([P, KB, TQ], dt.float8e4, tag=f"xh{q}",
                             name=f"xh{q}")
                nc.vector.tensor_copy(hi[:], stage[:])
                xhi[q] = hi
                return stage

            def load_quarter_lo(q, stage):
                """Convert the fp8 lo (residual) part of a token-quarter."""
                lo = xq.tile([P, KB, TQ], dt.float8e4, tag=f"xl{q}",
                             name=f"xl{q}")
                nc.vector.tensor_sub(lo[:], stage[:], xhi[q][:])
                xlo[q] = lo

            def load_chunk(ch):
                """DMA one W o-chunk (two halves) and ternarize into fp8."""
                o0 = ch * O_CHUNK
                wq = wqp.tile([P, KB, O_CHUNK], dt.float8e4, tag="wq",
                              name=f"wq{ch}")
                for h in range(2):
                    wf = wfp.tile([P, KB, O_CHUNK // 2], dt.float32, tag="wf",
                                  name=f"wf{ch}_{h}")
                    lo_o = o0 + h * (O_CHUNK // 2)
                    nc.sync.dma_start(
                        out=wf[:], in_=wT_r[:, :, lo_o:lo_o + O_CHUNK // 2])
                    nc.vector.tensor_scalar(wf[:], wf[:], 1.0, -1.0,
                                            mybir.AluOpType.min,
                                            mybir.AluOpType.max)
                    nc.vector.tensor_scalar(
                        wq[:, :, h * (O_CHUNK // 2):(h + 1) * (O_CHUNK // 2)],
                        wf[:], C_ROUND, C_ROUND,
                        mybir.AluOpType.add, mybir.AluOpType.subtract)
                return wq

            def cell(wq, ch, ob, tb):
                """One (chunk, o-block, token-quarter) accumulation: 32
                DoubleRow matmuls into a private PSUM bank, then evict with
                bias into bf16 and DMA the [128, 256] piece out."""
                o_abs = ch * O_CHUNK + ob * P
                lhs_lo = ob * P
                psum = ps.tile([P, 512], dt.float32, tag="ps",
                               name=f"ps_{ch}_{ob}_{tb}")
                for s in range(KB // 2):
                    for pi, xsrc in enumerate((xhi[tb], xlo[tb])):
                        nc.tensor.matmul(
                            psum[:, :TQ],
                            wq[:, 2 * s:2 * s + 2, lhs_lo:lhs_lo + P],
                            xsrc[:, 2 * s:2 * s + 2, :],
                            start=(s == 0 and pi == 0),
                            stop=(s == KB // 2 - 1 and pi == 1),
                            perf_mode=DR)
                stage = op.tile([P, TQ], dt.bfloat16, tag="out",
                                name=f"out_{ch}_{ob}_{tb}")
                nc.scalar.activation(
                    stage[:], psum[:, :TQ],
                    mybir.ActivationFunctionType.Identity,
                    bias=biasT[:, o_abs // P:o_abs // P + 1],
                    scale=1.0)
                nc.scalar.dma_start(
                    out=yT_d[o_abs:o_abs + P, tb * TQ:(tb + 1) * TQ],
                    in_=stage[:])

            rep_ctx = tc.For_i(0, reps, 1) if reps > 1 else contextlib.nullcontext()
            with rep_ctx:
              # x-frontloaded prologue: quarter 0 + chunk 0 first (earliest
              # possible PE start), then the remaining quarters at full DMA
              # bandwidth while chunk 0's cells run, then W chunks stream
              # back-to-back, each unlocking more PE work than its DMA time.
              load_quarter(0)
              wq0 = load_chunk(0)
              for tb in range(NQ):
                  if tb > 0:
                      load_quarter(tb)
                  for ob in range(O_CHUNK // P):
                      cell(wq0, 0, ob, tb)
              for ch in range(1, N_CHUNKS):
                  wq = load_chunk(ch)
                  for tb in range(NQ):
                      for ob in range(O_CHUNK // P):
                          cell(wq, ch, ob, tb)

    nc.compile()
    return nc


def kernel(input, weight, bias):
    global _built
    if _built is None:
        _built = _build()
    nc = _built
    from concourse.bass_utils import run_bass_kernel_spmd

    input = np.ascontiguousarray(input, dtype=np.float32)
    weight = np.ascontiguousarray(weight, dtype=np.float32)
    bias = np.ascontiguousarray(bias, dtype=np.float32)

    wT = np.ascontiguousarray(weight.T)                          # [i, o]
    biasT = np.ascontiguousarray(bias.reshape(OUT_F // P, P).T)  # [128, 32]

    in_maps = []
    for c in range(N_CORES):
        x_c = input[c * T_CORE:(c + 1) * T_CORE]                 # [1024, 4096]
        xT_c = np.ascontiguousarray(x_c.T)                       # [4096, 1024]
        in_maps.append({"xT": xT_c, "wT": wT, "biasT": biasT})

    res = run_bass_kernel_spmd(nc, in_maps, list(range(N_CORES)))

    y = np.empty((TOKENS, OUT_F), dtype=np.float32)
    for c in range(N_CORES):
        y[c * T_CORE:(c + 1) * T_CORE] = \
            np.asarray(res.results[c]["yT"]).astype(np.float32).T
    return y


# revision 21
# speedup vs baseline: 1.7045x; 1.0333x over previous
"""Ternary-weight linear layer on 8 Trainium2 NeuronCores.

Problem: y = x @ ternarize(W).T + b
  x [8192, 4096] fp32, W [4096, 4096] fp32, b [4096] fp32.
  ternarize(w) = round(clamp(w, -1, 1))  (round-half-even, forward value).

Strategy (data-parallel over tokens, replicated weights, fp8 DoubleRow):
  - Each of the 8 cores gets 1024 tokens. Host passes x and W transposed
    (layout-only prep) so the contraction dim i lands on SBUF partitions:
        xT  [4096 i, 1024 t]  (per-core slice)
        wT  [4096 i, 4096 o]  (replicated)
  - W is ternarized exactly on device with two DVE tensor_scalar ops:
    clamp via min/max (f32), then round-half-even via +C/-C (C = 1.5*2^23)
    writing float8e4 directly ({-1, 0, 1} are exact in fp8).
  - x is split on device into x_hi = fp8(x) and x_lo = fp8(x - x_hi).
    Two fp8 matmul passes accumulate into the same PSUM region; combined
    x quantization error is ~2^-8 relative (well under tolerance).
  - Matmuls use MatmulPerfMode.DoubleRow: lhsT [128, 2, 128] fp8 and
    rhs [128, 2, 256] fp8 contract TWO 128-deep k-slabs per instruction
    at 0.5 cycles per output column - 2x the bf16/f32r rate.
  - Pipeline: x quarters and the first W chunks are interleaved on one
    DMA queue (and in DVE issue order) so the PE starts early; each
    (chunk, o-block, token-quarter) cell accumulates in its own PSUM
    bank (8 banks rotating) and is evicted as soon as it stops.
  - Bias is added during PSUM->SBUF eviction on the scalar engine; the
    output is written as bf16 (halves the y DMA traffic; |rel err| ~2^-9).
  - Per-core output yT [4096 o, 1024 t] bf16; host casts + transposes.
"""

import contextlib

import numpy as np

N_CORES = 8
TOKENS = 8192
IN_F = 4096
OUT_F = 4096
T_CORE = TOKENS // N_CORES       # 1024 tokens per core
P = 128                          # partitions
KB = IN_F // P                   # 32 contraction slabs of 128
NQ = 4                           # x token quarters
TQ = T_CORE // NQ                # 256 tokens per quarter / per matmul
O_CHUNK = 256                    # o columns per W chunk
N_CHUNKS = OUT_F // O_CHUNK      # 16
C_ROUND = 12582912.0             # 1.5 * 2^23; (v+C)-C == round-half-even(v)

_built = None


def _build(reps=1):
    import concourse.bacc as bacc
    import concourse.mybir as mybir
    import concourse.tile as tile

    dt = mybir.dt
    DR = mybir.MatmulPerfMode.DoubleRow

    nc = bacc.Bacc("TRN2", target_bir_lowering=False, debug=False)
    xT_d = nc.dram_tensor("xT", [IN_F, T_CORE], dt.float32, kind="ExternalInput").ap()
    wT_d = nc.dram_tensor("wT", [IN_F, OUT_F], dt.float32, kind="ExternalInput").ap()
    biasT_d = nc.dram_tensor("biasT", [P, OUT_F // P], dt.float32,
                             kind="ExternalInput").ap()
    yT_d = nc.dram_tensor("yT", [OUT_F, T_CORE], dt.bfloat16,
                          kind="ExternalOutput").ap()

    xT_r = xT_d.rearrange("(kb p) t -> p kb t", p=P)     # [128, 32, 1024]
    wT_r = wT_d.rearrange("(kb p) o -> p kb o", p=P)     # [128, 32, 4096]

    with tile.TileContext(nc) as tc:
        with tc.tile_pool(name="xq", bufs=1) as xq, \
             tc.tile_pool(name="xf", bufs=4) as xf, \
             tc.tile_pool(name="wf", bufs=4) as wfp, \
             tc.tile_pool(name="wq", bufs=8) as wqp, \
             tc.tile_pool(name="op", bufs=8) as op, \
             tc.tile_pool(name="cn", bufs=1) as cn, \
             tc.tile_pool(name="ps", bufs=8, space="PSUM") as ps:

            biasT = cn.tile([P, OUT_F // P], dt.float32, name="biasT_s")
            nc.sync.dma_start(out=biasT[:], in_=biasT_d[:])

            # All streamed tensors are split into k-slab halves (16 slabs
            # each) as SEPARATE tiles: dependencies are tile-granular, so
            # halving the tiles halves every pipeline latency (DMA, convert,
            # first-matmul).  Matmuls s=0..7 read half 0, s=8..15 half 1.
            KH = KB // 2                       # 16 slabs per half
            xhi = [[None, None] for _ in range(NQ)]
            xlo = [[None, None] for _ in range(NQ)]

            def load_quarter_hi(q):
                """DMA one x token-quarter (two slab-halves) and convert
                the fp8 hi parts."""
                stages = []
                for sh in range(2):
                    stage = xf.tile([P, KH, TQ], dt.float32, tag="xf",
                                    name=f"xf{q}_{sh}")
                    nc.sync.dma_start(
                        out=stage[:],
                        in_=xT_r[:, sh * KH:(sh + 1) * KH,
                                 q * TQ:(q + 1) * TQ])
                    hi = xq.tile([P, KH, TQ], dt.float8e4, tag=f"xh{q}{sh}",
                                 name=f"xh{q}_{sh}")
                    nc.vector.tensor_copy(hi[:], stage[:])
                    xhi[q][sh] = hi
                    stages.append(stage)
                return stages

            def load_quarter_lo(q, stages):
                """Convert the fp8 lo (residual) parts of a token-quarter."""
                for sh in range(2):
                    lo = xq.tile([P, KH, TQ], dt.float8e4, tag=f"xl{q}{sh}",
                                 name=f"xl{q}_{sh}")
                    nc.vector.tensor_sub(lo[:], stages[sh][:],
                                         xhi[q][sh][:])
                    xlo[q][sh] = lo

            def load_chunk_half(ch, h):
                """DMA one W o-block (128 outs, two slab-halves) and
                ternarize into fp8 tiles."""
                lo_o = ch * O_CHUNK + h * P
                out = []
                for sh in range(2):
                    wf = wfp.tile([P, KH, P], dt.float32, tag="wf",
                                  name=f"wf{ch}_{h}_{sh}")
                    nc.sync.dma_start(
                        out=wf[:],
                        in_=wT_r[:, sh * KH:(sh + 1) * KH, lo_o:lo_o + P])
                    nc.vector.tensor_scalar(wf[:], wf[:], 1.0, -1.0,
                                            mybir.AluOpType.min,
                                            mybir.AluOpType.max)
                    wq = wqp.tile([P, KH, P], dt.float8e4, tag="wq",
                                  name=f"wq{ch}_{h}_{sh}")
                    nc.vector.tensor_scalar(wq[:], wf[:], C_ROUND, C_ROUND,
                                            mybir.AluOpType.add,
                                            mybir.AluOpType.subtract)
                    out.append(wq)
                return out

            def cell(wq, ch, ob, tb):
                """One (chunk, o-block, token-quarter) accumulation: 32
                DoubleRow matmuls into a private PSUM bank, then evict with
                bias into bf16 and DMA the [128, 256] piece out."""
                o_abs = ch * O_CHUNK + ob * P
                psum = ps.tile([P, 512], dt.float32, tag="ps",
                               name=f"ps_{ch}_{ob}_{tb}")
                # hi pass first, then lo: the lo conversion (DVE) hides
                # under the hi-pass matmuls
                for pi, xsrc in enumerate((xhi[tb], xlo[tb])):
                    for s in range(KB // 2):
                        sh, sl = divmod(s, KH // 2)
                        nc.tensor.matmul(
                            psum[:, :TQ],
                            wq[sh][:, 2 * sl:2 * sl + 2, :],
                            xsrc[sh][:, 2 * sl:2 * sl + 2, :],
                            start=(s == 0 and pi == 0),
                            stop=(s == KB // 2 - 1 and pi == 1),
                            perf_mode=DR)
                stage = op.tile([P, TQ], dt.bfloat16, tag="out",
                                name=f"out_{ch}_{ob}_{tb}")
                nc.scalar.activation(
                    stage[:], psum[:, :TQ],
                    mybir.ActivationFunctionType.Identity,
                    bias=biasT[:, o_abs // P:o_abs // P + 1],
                    scale=1.0)
                nc.scalar.dma_start(
                    out=yT_d[o_abs:o_abs + P, tb * TQ:(tb + 1) * TQ],
                    in_=stage[:])

            rep_ctx = tc.For_i(0, reps, 1) if reps > 1 else contextlib.nullcontext()
            with rep_ctx:
              # x-frontloaded prologue: quarter 0 + chunk 0 first (earliest
              # possible PE start), then the remaining quarters at full DMA
              # bandwidth while chunk 0's cells run, then W chunks stream
              # back-to-back, each unlocking more PE work than its DMA time.
              st0 = load_quarter_hi(0)
              wq00 = load_chunk_half(0, 0)
              load_quarter_lo(0, st0)
              wq01 = load_chunk_half(0, 1)
              wq0 = (wq00, wq01)
              for tb in range(NQ):
                  if tb > 0:
                      st = load_quarter_hi(tb)
                      load_quarter_lo(tb, st)
                  for ob in range(O_CHUNK // P):
                      cell(wq0[ob], 0, ob, tb)
              for ch in range(1, N_CHUNKS):
                  wqh = [load_chunk_half(ch, 0), load_chunk_half(ch, 1)]
                  for tb in range(NQ):
                      for ob in range(O_CHUNK // P):
                          cell(wqh[ob], ch, ob, tb)

    nc.compile()
    return nc


def kernel(input, weight, bias):
    global _built
    if _built is None:
        _built = _build()
    nc = _built
    from concourse.bass_utils import run_bass_kernel_spmd

    input = np.ascontiguousarray(input, dtype=np.float32)
    weight = np.ascontiguousarray(weight, dtype=np.float32)
    bias = np.ascontiguousarray(bias, dtype=np.float32)

    wT = np.ascontiguousarray(weight.T)                          # [i, o]
    biasT = np.ascontiguousarray(bias.reshape(OUT_F // P, P).T)  # [128, 32]

    in_maps = []
    for c in range(N_CORES):
        x_c = input[c * T_CORE:(c + 1) * T_CORE]                 # [1024, 4096]
        xT_c = np.ascontiguousarray(x_c.T)                       # [4096, 1024]
        in_maps.append({"xT": xT_c, "wT": wT, "biasT": biasT})

    res = run_bass_kernel_spmd(nc, in_maps, list(range(N_CORES)))

    y = np.empty((TOKENS, OUT_F), dtype=np.float32)
    for c in range(N_CORES):
        y[c * T_CORE:(c + 1) * T_CORE] = \
            np.asarray(res.results[c]["yT"]).astype(np.float32).T
    return y
